# revision 7
# baseline (speedup 1.0000x reference)
"""Trainium2 Bass kernel for nn_ConvolutionLayer_86595130622252.

Equivariant GNN message-passing layer:
  out0_a = radial_f0(x0)                      [N,N,O,1]
  out0_b = radial_f0(x1)                      [N,N,O,1]
  out1_a = unit(rij) x radial_f11(x0)         [N,N,O,3]
  out1_b = unit(rij) x radial_f10(x1)         [N,N,O,3]
  out1_c = unit(rij) x radial_f11(x1)         [N,N,O,3]
where radial_f(x) = relu(x @ w1 + b1) @ w2 + b2.

Strategy (8 NeuronCores, data-parallel over the first node axis):
  * each core owns 64 rows -> M = 64*512 = 32768 (i,j) pairs
  * host packs xx = [x0_shard.T ; x1_shard.T] as a [128, M] f32 array so the
    contraction dim (features of both tensors) lies on SBUF partitions: zero
    on-chip transposes, fully contiguous DMA.
  * layer-1: 3 matmuls per 512-pair chunk with block/stacked weights
    (K=128 spans x0+x1 feature rows; two filters packed along lhsT columns),
    fp32r for full PE rate at N=512.
  * relu+b1 on ScalarE (bias is per-partition: partition = hidden unit).
  * layer-2 swaps operand roles: lhsT = hidden slice [128, 128pairs],
    rhs = block-diag w2 -> PSUM output is [128 pairs, rad] i.e. already in the
    pair-major layout the output wants.
  * DVE adds b2 (row-replicated const) straight out of PSUM and forms the
    u (x) rad outer product with 0-stride broadcast access patterns.
  * outputs accumulate in SBUF and are stored with >=2KB/partition contiguous
    runs; the host un-permutes the blocked layout (layout only, no math).
"""

import os
import sys
from contextlib import ExitStack

import numpy as np

for _p in ("/opt/trn_rl_repo", "/root/.axon_site/_ro/trn_rl_repo"):
    if os.path.isdir(_p) and _p not in sys.path:
        sys.path.insert(0, _p)

import concourse.bacc as bacc  # noqa: E402
import concourse.bass as bass  # noqa: E402
import concourse.tile as tile  # noqa: E402
from concourse import mybir  # noqa: E402
from concourse.bass_utils import run_bass_kernel_spmd  # noqa: E402

# problem sizes (hardcoded per spec)
N = 512
D = 64
H = 64
O = 32
NCORES = 8
ROWS = N // NCORES          # 64 node rows per core
M = ROWS * N                # 32768 pairs per core
P = 128                     # pairs per subtile (SBUF partitions)
SUP = 2048                  # pairs per supertile
NSUP = M // SUP             # 16 supertiles
CHUNK = 512                 # pairs per chunk (one L1 matmul free dim)
NCHUNK = SUP // CHUNK       # 4 chunks per supertile
NT = CHUNK // P             # 4 subtiles per chunk
KSUB = SUP // P             # 16 subtiles per supertile
EPS = 1e-8

F32 = mybir.dt.float32
F32R = mybir.dt.float32r


def _build_nc(m_pairs: int):
    """Build the per-core Bass program for m_pairs pairs (m_pairs % SUP == 0)."""
    nsup = m_pairs // SUP
    nc = bacc.Bacc("TRN2", target_bir_lowering=False, debug=False)

    # ---- DRAM I/O ----
    xx = nc.dram_tensor("xx", [P, m_pairs], F32R, kind="ExternalInput").ap()
    rij = nc.dram_tensor("rij", [P, nsup * KSUB * 3], F32, kind="ExternalInput").ap()
    cpr = nc.dram_tensor("cpr", [P, 320], F32R, kind="ExternalInput").ap()
    cp = nc.dram_tensor("cp", [P, 259], F32, kind="ExternalInput").ap()

    o0a = nc.dram_tensor("o0a", [nsup, P, KSUB * O], F32, kind="ExternalOutput").ap()
    o0b = nc.dram_tensor("o0b", [nsup, P, KSUB * O], F32, kind="ExternalOutput").ap()
    o1a = nc.dram_tensor("o1a", [nsup, P, KSUB * O * 3], F32, kind="ExternalOutput").ap()
    o1b = nc.dram_tensor("o1b", [nsup, P, KSUB * O * 3], F32, kind="ExternalOutput").ap()
    o1c = nc.dram_tensor("o1c", [nsup, P, KSUB * O * 3], F32, kind="ExternalOutput").ap()

    with tile.TileContext(nc) as tc, ExitStack() as st:
        consts = st.enter_context(tc.tile_pool(name="consts", bufs=1))
        xxp = st.enter_context(tc.tile_pool(name="xxp", bufs=2))
        hp = st.enter_context(tc.tile_pool(name="hp", bufs=2))
        up = st.enter_context(tc.tile_pool(name="up", bufs=2))
        ob = st.enter_context(tc.tile_pool(name="ob", bufs=2))
        ps1 = st.enter_context(tc.tile_pool(name="ps1", bufs=1, space="PSUM"))
        ps2 = st.enter_context(tc.tile_pool(name="ps2", bufs=2, space="PSUM"))

        # resident constants (two packed loads to keep dep fan-in small)
        cpr_t = consts.tile([P, 320], F32R)
        cp_t = consts.tile([P, 259], F32)
        srij = consts.tile([P, nsup * KSUB * 3], F32)
        seps = consts.tile([P, 1], F32)
        nc.sync.dma_start(out=cpr_t, in_=cpr)
        nc.sync.dma_start(out=cp_t, in_=cp)
        nc.sync.dma_start(out=srij, in_=rij)
        nc.vector.memset(seps, EPS)
        sw1a, sw1b, sw1c = cpr_t[:, 0:128], cpr_t[:, 128:256], cpr_t[:, 256:320]
        sb1a, sb1b, sb1c = cp_t[:, 0:1], cp_t[:, 1:2], cp_t[0:64, 2:3]
        sw2a, sw2b, sw2c = cp_t[:, 3:67], cp_t[:, 67:131], cp_t[0:64, 131:163]
        srb0, srb10, srb11 = cp_t[:, 163:195], cp_t[:, 195:227], cp_t[:, 227:259]

        def bcast_o3(ap_po):
            # [P, t, O] -> [P, t, O, 3] via 0-stride last dim
            return ap_po.unsqueeze(3).to_broadcast([ap_po.shape[0], ap_po.shape[1], O, 3])

        for s in range(nsup):
            xx_t = xxp.tile([P, SUP], F32R, tag="xx")
            nc.sync.dma_start(out=xx_t, in_=xx[:, s * SUP:(s + 1) * SUP])

            # --- u = rij / sqrt(sum(rij^2) + eps) for this supertile ---
            rs = srij[:, s * KSUB * 3:(s + 1) * KSUB * 3]      # [P, 48]
            rs3 = rs.rearrange("p (k c) -> p k c", c=3)        # [P, 16, 3]
            sq = up.tile([P, KSUB, 3], F32, tag="sq")
            nc.vector.tensor_mul(sq, rs3, rs3)
            ssum = up.tile([P, KSUB], F32, tag="ssum")
            nc.vector.tensor_add(ssum, sq[:, :, 0], sq[:, :, 1])
            ssum2 = up.tile([P, KSUB], F32, tag="ssum2")
            nc.vector.tensor_add(ssum2, ssum, sq[:, :, 2])
            srt = up.tile([P, KSUB], F32, tag="srt")
            nc.scalar.activation(srt, ssum2, mybir.ActivationFunctionType.Sqrt,
                                 bias=seps, scale=1.0)
            rec = up.tile([P, KSUB], F32, tag="rec")
            nc.vector.reciprocal(rec, srt)
            u_t = up.tile([P, KSUB, 3], F32, tag="u")
            nc.vector.tensor_mul(
                u_t, rs3,
                rec.unsqueeze(2).to_broadcast([P, KSUB, 3]),
            )

            # output accumulation buffers for this supertile
            buf0a = ob.tile([P, KSUB * O], F32, tag="buf0a")
            buf0b = ob.tile([P, KSUB * O], F32, tag="buf0b")
            bufr3 = ob.tile([P, KSUB * O], F32, tag="bufr3")
            bufr4 = ob.tile([P, KSUB * O], F32, tag="bufr4")
            bufr5 = ob.tile([P, KSUB * O], F32, tag="bufr5")
            buf1a = ob.tile([P, KSUB * O * 3], F32, tag="buf1a")
            buf1b = ob.tile([P, KSUB * O * 3], F32, tag="buf1b")
            buf1c = ob.tile([P, KSUB * O * 3], F32, tag="buf1c")

            for c in range(NCHUNK):
                xs = xx_t[:, c * CHUNK:(c + 1) * CHUNK]
                # ---- layer 1: hiddenT = (w1 pack).T @ xx  [hid, pairs] ----
                hA_ps = ps1.tile([P, CHUNK], F32, tag="hA_ps")
                hB_ps = ps1.tile([P, CHUNK], F32, tag="hB_ps")
                hC_ps = ps1.tile([64, CHUNK], F32, tag="hC_ps")
                nc.tensor.matmul(hA_ps, sw1a, xs, start=True, stop=True)
                nc.tensor.matmul(hB_ps, sw1b, xs, start=True, stop=True)
                nc.tensor.matmul(hC_ps, sw1c, xs, start=True, stop=True)
                hA = hp.tile([P, CHUNK], F32, tag="hA")
                hB = hp.tile([P, CHUNK], F32, tag="hB")
                hC = hp.tile([64, CHUNK], F32, tag="hC")
                nc.scalar.activation(hA, hA_ps, mybir.ActivationFunctionType.Relu,
                                     bias=sb1a, scale=1.0)
                nc.scalar.activation(hB, hB_ps, mybir.ActivationFunctionType.Relu,
                                     bias=sb1b, scale=1.0)
                nc.scalar.activation(hC, hC_ps, mybir.ActivationFunctionType.Relu,
                                     bias=sb1c, scale=1.0)

                # ---- layer 2: rad[pairs, o] = hiddenT.T @ w2 packs ----
                l2ab = ps2.tile([P, NT * 128], F32, tag="l2ab")
                l2c = ps2.tile([P, NT * 32], F32, tag="l2c")
                for t in range(NT):
                    hsl = slice(t * P, (t + 1) * P)
                    nc.tensor.matmul(l2ab[:, t * 128:t * 128 + 64],
                                     hA[:, hsl], sw2a, start=True, stop=True)
                    nc.tensor.matmul(l2ab[:, t * 128 + 64:t * 128 + 128],
                                     hB[:, hsl], sw2b, start=True, stop=True)
                    nc.tensor.matmul(l2c[:, t * 32:(t + 1) * 32],
                                     hC[:, hsl], sw2c, start=True, stop=True)

                # views [P, t, sect, O] of the layer-2 psum
                abv = l2ab.rearrange("p (t s o) -> p t s o", t=NT, s=4, o=O)
                cv = l2c.rearrange("p (t o) -> p t o", t=NT, o=O)
                csl = slice(c * NT * O, (c + 1) * NT * O)
                b0av = buf0a[:, csl].rearrange("p (t o) -> p t o", t=NT, o=O)
                b0bv = buf0b[:, csl].rearrange("p (t o) -> p t o", t=NT, o=O)
                br3v = bufr3[:, csl].rearrange("p (t o) -> p t o", t=NT, o=O)
                br4v = bufr4[:, csl].rearrange("p (t o) -> p t o", t=NT, o=O)
                br5v = bufr5[:, csl].rearrange("p (t o) -> p t o", t=NT, o=O)

                def rep_t(bias_tile):
                    return bias_tile.unsqueeze(1).to_broadcast([P, NT, O])

                nc.vector.tensor_add(b0av, abv[:, :, 0], rep_t(srb0))
                nc.vector.tensor_add(br3v, abv[:, :, 1], rep_t(srb11))
                nc.vector.tensor_add(b0bv, abv[:, :, 2], rep_t(srb0))
                nc.vector.tensor_add(br4v, abv[:, :, 3], rep_t(srb10))
                nc.vector.tensor_add(br5v, cv, rep_t(srb11))

                # ---- out1 = u (x) rad ----
                uc = u_t[:, c * NT:(c + 1) * NT, :]            # [P, NT, 3]
                ucb = uc.unsqueeze(2).to_broadcast([P, NT, O, 3])
                c3sl = slice(c * NT * O * 3, (c + 1) * NT * O * 3)
                for buf1, radv in ((buf1a, br3v), (buf1b, br4v), (buf1c, br5v)):
                    ov = buf1[:, c3sl].rearrange(
                        "p (t o c3) -> p t o c3", t=NT, o=O, c3=3)
                    nc.vector.tensor_mul(ov, bcast_o3(radv), ucb)

            # ---- store supertile ----
            nc.sync.dma_start(out=o0a[s], in_=buf0a)
            nc.sync.dma_start(out=o0b[s], in_=buf0b)
            nc.sync.dma_start(out=o1a[s], in_=buf1a)
            nc.sync.dma_start(out=o1b[s], in_=buf1b)
            nc.sync.dma_start(out=o1c[s], in_=buf1c)

    nc.compile()
    return nc


def _tf32_round(a):
    """Round fp32 -> tf32 (10-bit mantissa, round-to-nearest-even) in fp32 bits."""
    u = np.ascontiguousarray(a, np.float32).view(np.uint32)
    u = (u + 0x0FFF + ((u >> 13) & 1)) & np.uint32(0xFFFFE000)
    return u.view(np.float32)


def _pack_host_inputs(x0, x1, rbf, rij, weights, m_pairs=M, ncores=NCORES):
    """Shard + pack full inputs -> list of per-core input dicts."""
    del rbf  # unused by the math
    rows = x0.shape[0] // ncores
    (f0_w1, f0_b1, f0_w2, f0_b2,
     f10_w1, f10_b1, f10_w2, f10_b2,
     f11_w1, f11_b1, f11_w2, f11_b2) = weights

    f32 = np.float32
    w1a = np.zeros((128, 128), f32); w1a[0:64, 0:64] = f0_w1; w1a[0:64, 64:128] = f11_w1
    w1b = np.zeros((128, 128), f32); w1b[64:128, 0:64] = f0_w1; w1b[64:128, 64:128] = f10_w1
    w1c = np.zeros((128, 64), f32); w1c[64:128, 0:64] = f11_w1
    b1a = np.concatenate([f0_b1, f11_b1]).astype(f32).reshape(128, 1)
    b1b = np.concatenate([f0_b1, f10_b1]).astype(f32).reshape(128, 1)
    b1c = f11_b1.astype(f32).reshape(64, 1)
    w2a = np.zeros((128, 64), f32); w2a[0:64, 0:32] = f0_w2; w2a[64:128, 32:64] = f11_w2
    w2b = np.zeros((128, 64), f32); w2b[0:64, 0:32] = f0_w2; w2b[64:128, 32:64] = f10_w2
    w2c = f11_w2.astype(f32)
    rb2_0 = np.broadcast_to(f0_b2, (128, 32)).astype(f32).copy()
    rb2_10 = np.broadcast_to(f10_b2, (128, 32)).astype(f32).copy()
    rb2_11 = np.broadcast_to(f11_b2, (128, 32)).astype(f32).copy()

    cpr = _tf32_round(np.concatenate([w1a, w1b, w1c], axis=1))
    cp = np.zeros((128, 259), f32)
    cp[:, 0:1] = b1a; cp[:, 1:2] = b1b; cp[0:64, 2:3] = b1c
    cp[:, 3:67] = w2a; cp[:, 67:131] = w2b; cp[0:64, 131:163] = w2c
    cp[:, 163:195] = rb2_0; cp[:, 195:227] = rb2_10; cp[:, 227:259] = rb2_11
    consts = dict(cpr=cpr, cp=cp)

    nsup = m_pairs // SUP
    in_maps = []
    for i in range(ncores):
        r0, r1 = i * rows, (i + 1) * rows
        x0s = x0[r0:r1].reshape(-1, D)[:m_pairs]        # [M, 64]
        x1s = x1[r0:r1].reshape(-1, D)[:m_pairs]
        xxi = _tf32_round(np.concatenate([
            np.ascontiguousarray(x0s.T), np.ascontiguousarray(x1s.T)
        ], axis=0).astype(f32))                         # [128, M]
        rijs = rij[r0:r1].reshape(-1, 3)[:m_pairs]
        rijp = np.ascontiguousarray(
            rijs.reshape(nsup, KSUB, P, 3).transpose(2, 0, 1, 3)
        ).reshape(P, nsup * KSUB * 3).astype(f32)
        in_maps.append(dict(xx=xxi, rij=rijp, **consts))
    return in_maps


def _unpack_core_outputs(res, m_pairs=M):
    """Device blocked layout -> [M, O, ...] pair-major arrays for one core."""
    nsup = m_pairs // SUP
    o0a = res["o0a"].reshape(nsup, P, KSUB, O).transpose(0, 2, 1, 3).reshape(m_pairs, O, 1)
    o0b = res["o0b"].reshape(nsup, P, KSUB, O).transpose(0, 2, 1, 3).reshape(m_pairs, O, 1)
    o1a = res["o1a"].reshape(nsup, P, KSUB, O, 3).transpose(0, 2, 1, 3, 4).reshape(m_pairs, O, 3)
    o1b = res["o1b"].reshape(nsup, P, KSUB, O, 3).transpose(0, 2, 1, 3, 4).reshape(m_pairs, O, 3)
    o1c = res["o1c"].reshape(nsup, P, KSUB, O, 3).transpose(0, 2, 1, 3, 4).reshape(m_pairs, O, 3)
    return o0a, o0b, o1a, o1b, o1c


_NC_CACHE = {}


def _get_nc(m_pairs):
    if m_pairs not in _NC_CACHE:
        _NC_CACHE[m_pairs] = _build_nc(m_pairs)
    return _NC_CACHE[m_pairs]


def kernel(x0, x1, rbf, rij,
           f0_w1, f0_b1, f0_w2, f0_b2,
           f10_w1, f10_b1, f10_w2, f10_b2,
           f11_w1, f11_b1, f11_w2, f11_b2,
           trace=False):
    weights = (f0_w1, f0_b1, f0_w2, f0_b2,
               f10_w1, f10_b1, f10_w2, f10_b2,
               f11_w1, f11_b1, f11_w2, f11_b2)
    weights = tuple(np.asarray(w, np.float32) for w in weights)
    x0 = np.asarray(x0, np.float32)
    x1 = np.asarray(x1, np.float32)
    rij = np.asarray(rij, np.float32)

    in_maps = _pack_host_inputs(x0, x1, rbf, rij, weights)
    nc = _get_nc(M)
    out = run_bass_kernel_spmd(nc, in_maps, list(range(NCORES)), trace=trace)

    rows = ROWS
    parts = [[] for _ in range(5)]
    for i in range(NCORES):
        core_outs = _unpack_core_outputs(out.results[i])
        for j, arr in enumerate(core_outs):
            last = arr.shape[-1]
            parts[j].append(arr.reshape(rows, N, O, last))
    full = [np.concatenate(p, axis=0).astype(np.float32) for p in parts]
    result = tuple(full)
    kernel.last_exec_time_ns = out.exec_time_ns
    return result


kernel.last_exec_time_ns = None
